# revision 55
# baseline (speedup 1.0000x reference)
"""Trainium2 Bass kernel for the Laplace-kernel feature expansion.

Reference computation (per scalar x of the [16, 64, 64, 64] input):
    phi_i  = exp(-|x - p_i|)            for 15 design points p_i
    out_j  = sum_i chol_inv[i, j] phi_i
scattered so out[b, c*15 + j, h, w] comes from x[b, c, h, w].

Distribution: pure data parallel, 2 batches per core across 8 cores.

Per-core dataflow (no collectives):
  1. x is pre-split on host into bf16 (hi, lo) pairs, laid out so a
     graduated sequence of full-width DMAs loads the whole per-core
     input into SBUF once; the stationary weights (w4, block-diag
     chol_inv) ride in the first 256 columns of the same stream so one
     leading DMA delivers weights plus the first spatial chunk.  A dummy
     exp on the weight columns pulls the ScalarE activation table set
     (~2.7us) off the critical path while x streams in.
  2. TensorE "broadcast" matmuls with a 0/1 block matrix replicate each
     x value onto 15 partitions (8 channel groups x 15 = 120 partitions),
     reconstructing fp32 x = hi + lo in PSUM; an extra ones-row makes the
     same matmul subtract the design point p_i (p_i exact in bf16).
     The K=17 matmuls are packed 4x into the 128x128 array via
     tile_position row-tiling (4 concurrent quadrant matmuls).
  3. VectorE computes |T| in one op (sign-bit clear on an int32 view).
  4. ScalarE computes exp(-|t|) -> bf16.
  5. TensorE applies block-diag(chol_inv) -> PSUM (fp32).
  6. PSUM evicted to SBUF (split between ScalarE/VectorE; the 55/128
     interleave is an empirically tuned balance/latency pattern), then
     streamed to DRAM in [120, 1024] pieces as soon as each half-chunk's
     evictions land, overlapping output DMA with the next chunk.

Engine budget per core (measured): DVE abs 64x~1.22us + ~55 evict casts
~0.68us; ScalarE exp 64x~1.0us + ~73 evict copies ~0.8us; both engines
~90% busy and jointly the bottleneck -- PSUM-sourced fp32 elementwise
ops run at 1 elem/lane/cycle on TRN2 (16-bit PSUM matmul output, which
would unlock the DVE 2x mode, is TRN3-only).

Spatial mapping: PE-array quadrant q = 2j+l covers, within a (b, cblock)
tile, the spatial columns 2048j + 1024h + 512l + c (h = half), so each
post-projection PSUM chunk evicts to a contiguous 1024-column span.
"""

import sys

if "/opt/trn_rl_repo" not in sys.path:
    sys.path.insert(0, "/opt/trn_rl_repo")

import numpy as np
import ml_dtypes


def _ensure_axon_hooks_stub():
    """run_bass_kernel_spmd imports antenv.axon_hooks when BASS_TRACE is
    set; the module is absent on some images. Provide a no-op stub so a
    stray BASS_TRACE env var cannot crash the kernel (tracing is then
    skipped gracefully)."""
    try:
        import antenv.axon_hooks  # noqa: F401
    except ImportError:
        import types

        try:
            import antenv
        except ImportError:
            return
        mod = types.ModuleType("antenv.axon_hooks")
        _hook = [None]
        mod.set_axon_ntff_profile_hook = lambda h: _hook.__setitem__(0, h)
        mod.get_axon_ntff_profile_hook = lambda: _hook[0]
        sys.modules["antenv.axon_hooks"] = mod
        antenv.axon_hooks = mod


_ensure_axon_hooks_stub()

BF16 = ml_dtypes.bfloat16

B, C, H, W = 16, 64, 64, 64
P = H * W                # 4096 spatial positions
M_PTS = 15               # design points
G = 8                    # channel groups per tile
MROWS = G * M_PTS        # 120 partitions used
KIN = 2 * G + 1          # 17 moving rows for the broadcast matmul
NCORES = 8
BPC = B // NCORES        # batches per core (2)
CBLK = C // G            # channel-block tiles per batch (8)
NTILES = BPC * CBLK      # 16 (b, cblock) tiles per core
QCOLS = NTILES * 1024    # 16384 columns per quadrant row

# Engine balance (measured cost model): ScalarE does all 64 exps plus
# SCE_ABS_NUM of the 64 abs ops (ScalarE Abs 997ns vs DVE 1222ns per
# [*,1024] chunk); VectorE does the remaining abs ops plus all 128
# [*,512] evictions (DVE cast 683ns vs ScalarE copy 804ns).
DVE_EVICT_NUM = 55
TOTAL_EVICTS = 128
SCE_ABS_NUM = 0
TOTAL_ABS = 64
WCOLS = 256              # w4 + r_blk packed ahead of x in the DMA stream

_CACHED = {}


def _build_nc():
    from concourse import bacc
    import concourse.mybir as mybir
    from concourse.tile import TileContext

    dt = mybir.dt
    Act = mybir.ActivationFunctionType
    Alu = mybir.AluOpType

    nc = bacc.Bacc(
        "TRN2", target_bir_lowering=False, debug=False, num_devices=NCORES
    )
    x_full = nc.declare_dram_parameter(
        "x_full", [128, WCOLS + QCOLS], dt.bfloat16, isOutput=False
    )
    out = nc.declare_dram_parameter(
        "out", [BPC, C * M_PTS, P], dt.bfloat16, isOutput=True
    )

    with TileContext(nc) as tc:
        with (
            tc.tile_pool(name="const", bufs=1) as cpool,
            tc.tile_pool(name="xbig", bufs=1) as xpool,
            tc.tile_pool(name="absT", bufs=4) as apool,
            tc.tile_pool(name="phi", bufs=6) as ppool,
            tc.tile_pool(name="osb", bufs=6) as opool,
            tc.tile_pool(name="psT", bufs=1, space="PSUM") as psTp,
            tc.tile_pool(name="psO", bufs=2, space="PSUM") as psOp,
        ):
            # Whole per-core input resident in SBUF (32 KB/partition).
            # The stationary weights (w4, block-diag chol_inv) ride in the
            # first WCOLS columns of the same buffer so a single leading
            # DMA delivers weights + the first spatial chunk; graduated
            # full-width DMAs stream the rest behind it.
            xbig = xpool.tile([128, WCOLS + QCOLS], dt.bfloat16)
            nc.sync.dma_start(out=xbig[:, 0 : WCOLS + 512], in_=x_full[:, 0 : WCOLS + 512])
            # Pull the exp table set into ScalarE's table RAMs (~2.7us)
            # while the bulk of x is still streaming in.
            scr = cpool.tile([128, 16], dt.bfloat16)
            nc.scalar.activation(scr[0:1, :], xbig[0:1, 0:16], Act.Exp, scale=-1.0)
            pos = WCOLS + 512
            for width in (1024, 2048, 4096, 8704):
                nc.sync.dma_start(
                    out=xbig[:, pos : pos + width], in_=x_full[:, pos : pos + width]
                )
                pos += width

            # Per-chunk software pipeline with a +1 broadcast skew: the
            # broadcast matmuls for chunk k+1 are emitted BEFORE the
            # projection matmuls for chunk k, so when proj(k) blocks the
            # strict-FIFO PE queue waiting on exp(k), the next chunk's
            # broadcasts are already ahead of it and the T->abs->exp
            # chain keeps running (closes ~400-600ns/chunk ScalarE gaps).
            NCHUNK = NTILES * 4  # chunk k = (t, h, j), 1024 cols each

            def emit_bcast(k):
                t, r = divmod(k, 4)
                h, j = divmod(r, 2)
                Tt = psTp.tile(
                    [128, 1024], dt.float32, name=f"tps{k % 3}", tag=f"tps{k % 3}"
                )
                for l in range(2):
                    q = 2 * j + l
                    nc.tensor.matmul(
                        Tt[:, l * 512 : (l + 1) * 512],
                        xbig[32 * q : 32 * q + KIN, 0:128],
                        xbig[
                            32 * q : 32 * q + KIN,
                            WCOLS + t * 1024 + h * 512 : WCOLS
                            + t * 1024
                            + (h + 1) * 512,
                        ],
                        start=True,
                        stop=True,
                        tile_position=(32 * q, 0),
                    )
                return Tt

            gc = 0
            ot = None
            Tnext = emit_bcast(0)
            for k in range(NCHUNK):
                t, r = divmod(k, 4)
                h, j = divmod(r, 2)
                b, cb = divmod(t, CBLK)
                if k % 4 == 0:
                    ot = opool.tile([MROWS, P], dt.bfloat16)
                Tt = Tnext
                if k + 1 < NCHUNK:
                    Tnext = emit_bcast(k + 1)
                # |T| via sign-bit clear on an int32 view (DVE), in place
                # in PSUM so exp reads via ScalarE's faster PSUM port
                nc.vector.tensor_scalar(
                    out=Tt[0:MROWS, :].bitcast(dt.int32),
                    in0=Tt[0:MROWS, :].bitcast(dt.int32),
                    scalar1=0x7FFFFFFF,
                    scalar2=None,
                    op0=Alu.bitwise_and,
                )
                pt = ppool.tile([MROWS, 1024], dt.bfloat16, name=f"pt{k % 3}")
                nc.scalar.activation(pt[:], Tt[0:MROWS, :], Act.Exp, scale=-1.0)
                base = 2048 * j + 1024 * h
                for l in range(2):
                    ops = psOp.tile([128, 512], dt.float32)
                    nc.tensor.matmul(
                        ops[:],
                        xbig[0:MROWS, 128:256],
                        pt[:, l * 512 : (l + 1) * 512],
                        start=True,
                        stop=True,
                    )
                    dst = ot[:, base + 512 * l : base + 512 * (l + 1)]
                    if (gc * DVE_EVICT_NUM) % TOTAL_EVICTS < DVE_EVICT_NUM:
                        nc.vector.tensor_copy(out=dst, in_=ops[0:MROWS, :])
                    else:
                        nc.scalar.activation(dst, ops[0:MROWS, :], Act.Copy)
                    gc += 1
                # stream this chunk's contiguous 1024-col span out as soon
                # as its evictions land
                nc.sync.dma_start(
                    out=out[b, cb * MROWS : (cb + 1) * MROWS, base : base + 1024],
                    in_=ot[:, base : base + 1024],
                )
    nc.compile()
    return nc


def _host_prep(x, design_points, chol_inv):
    """Build the derived host-side arrays fed to the device."""
    pts = np.asarray(design_points, dtype=np.float32)
    xs = np.ascontiguousarray(np.asarray(x, dtype=np.float32)).reshape(B, C, P)
    x_hi = xs.astype(BF16)
    x_lo = (xs - x_hi.astype(np.float32)).astype(BF16)

    # spatial = 2048j + 1024h + 512l + c ; quadrant q = 2j + l
    # arr[q, r, b, cb, h, c(512)] with r = 2g + part (hi/lo), r=16 -> 1.0
    def to_quad(a):  # [B, C, P] -> [4(q), G, B, CBLK, 2(h), 512]
        a7 = a.reshape(B, CBLK, G, 2, 2, 2, 512)  # [b, cb, g, j, h, l, c]
        return a7.transpose(3, 5, 2, 0, 1, 4, 6).reshape(4, G, B, CBLK, 2, 512)

    arr = np.empty((4, KIN, B, CBLK, 2, 512), dtype=BF16)
    arr[:, 0 : 2 * G : 2] = to_quad(x_hi)
    arr[:, 1 : 2 * G : 2] = to_quad(x_lo)
    arr[:, 2 * G] = BF16(1.0)

    w17 = np.zeros((KIN, 128), dtype=np.float32)
    for g in range(G):
        w17[2 * g, 15 * g : 15 * g + 15] = 1.0
        w17[2 * g + 1, 15 * g : 15 * g + 15] = 1.0
        w17[2 * G, 15 * g : 15 * g + 15] = -pts
    w4 = np.zeros((128, 128), dtype=np.float32)
    for q in range(4):
        w4[32 * q : 32 * q + KIN] = w17
    w4 = w4.astype(BF16)

    chol = np.asarray(chol_inv, dtype=np.float32)
    r_blk = np.zeros((MROWS, 128), dtype=np.float32)
    for g in range(G):
        r_blk[15 * g : 15 * g + 15, 15 * g : 15 * g + 15] = chol
    r_blk = r_blk.astype(BF16)

    return arr, w4, r_blk


LAST_RESULT = None


def kernel(x, design_points, chol_inv):
    global LAST_RESULT
    from concourse.bass_utils import run_bass_kernel_spmd

    if "nc" not in _CACHED:
        _CACHED["nc"] = _build_nc()
    nc = _CACHED["nc"]

    arr, w4, r_blk = _host_prep(x, design_points, chol_inv)

    in_maps = []
    for core in range(NCORES):
        # per-core [4, 17, 16384] placed into a [128, 16384] buffer at
        # partition offsets 32q (rows 17..31 of each quadrant unused);
        # weights packed into the leading WCOLS columns.
        x_q = arr[:, :, core * BPC : (core + 1) * BPC].reshape(4, KIN, QCOLS)
        x_full = np.zeros((128, WCOLS + QCOLS), dtype=BF16)
        x_full[:, 0:128] = w4
        x_full[0:MROWS, 128:256] = r_blk
        for q in range(4):
            x_full[32 * q : 32 * q + KIN, WCOLS:] = x_q[q]
        in_maps.append({"x_full": x_full})

    res = run_bass_kernel_spmd(nc, in_maps, core_ids=list(range(NCORES)))
    LAST_RESULT = res

    full = np.empty((B, C * M_PTS, P), dtype=np.float32)
    for core in range(NCORES):
        full[core * BPC : (core + 1) * BPC] = res.results[core]["out"]
    return full.reshape(B, C * M_PTS, H, W)

